# revision 1
# baseline (speedup 1.0000x reference)
"""Trainium2 Bass kernel for nn_DSRLossStateless (DSR loss, stateless).

loss = -sum_t(D_t)/B where D_t comes from an eta-EMA pair (A,B) over
portfolio returns R_t = sum_a w[t,a]*nr[t,a].

Strategy (8 cores, batch-sharded):
  - Each core owns 250k consecutive rows, laid out as SBUF partitions
    1..125 x 2000 columns (time-major within a partition). Partition 0
    holds the 2000 rows preceding the core's span (synthetic for core 0),
    which supplies the scan carry for partition 1.
  - Stage A (memory-bound bulk): tiled DMA of w/nr, elementwise product
    (split GPSIMD/DVE), segmented rowsum-of-16 on DVE -> R. Per chunk,
    ACT derives eta*R, eta*R^2, R^2 and DVE extends chained local scans
    (zero-carry) for A_loc/B_loc - all overlapped with the DMA stream.
  - Tail: per-partition carries are just the previous partition's local
    scan final (c^2000 ~ 1.9e-9 kills older terms), applied as
    A_prev = A_loc_shift + carry*c^t in one fused op per scan; then a
    short elementwise chain computes q_t = -D_t/eta and fused reduces
    leave one partial per core.
  - Host: loss = eta * sum(partials) / B.
"""

import sys

sys.path.insert(0, "/opt/trn_rl_repo")

import numpy as np

import concourse.bass as bass
import concourse.bacc as bacc
import concourse.tile as tile
from concourse import mybir
from concourse.bass_utils import run_bass_kernel_spmd
from contextlib import ExitStack

F32 = mybir.dt.float32
NF32 = np.float32

N_CORES = 8
NA = 16            # assets (inner dim)
KP = 126           # SBUF partitions used (0 = prepend/carry-feeder)
L = 2000           # columns (time steps per partition)
LE = L + 1         # local-scan buffer width (col 0 = zero carry)
OWN = (KP - 1) * L      # rows owned per core = 250000
B_TOTAL = N_CORES * OWN # 2000000
CH = 16            # stage-A chunks
KC = L // CH       # 125 rows per partition per chunk
FW = KC * NA       # 2000 f32 per partition per chunk tile
ETA = 0.01
EPS = 1e-8
CDEC = NF32(1.0 - ETA)  # 0.99

AL = mybir.AluOpType
AF = mybir.ActivationFunctionType
AX = mybir.AxisListType

_PROGRAM = None


def _build_program():
    nc = bacc.Bacc("TRN2", target_bir_lowering=False, debug=False)

    w_ap = nc.dram_tensor("w", [KP * L, NA], F32, kind="ExternalInput").ap()
    nr_ap = nc.dram_tensor("nr", [KP * L, NA], F32, kind="ExternalInput").ap()
    out_ap = nc.dram_tensor("out", [1, 1], F32, kind="ExternalOutput").ap()

    # geo_c[t] = c^t (carry decay for the correction pass)
    geoc_np = (CDEC ** np.arange(L).astype(NF32)).astype(NF32)
    geoc_dram = nc.inline_tensor(
        np.ascontiguousarray(np.broadcast_to(geoc_np, (KP, L))), name="geoc"
    )

    w_v = w_ap.rearrange("(p t) a -> p (t a)", p=KP)
    nr_v = nr_ap.rearrange("(p t) a -> p (t a)", p=KP)

    with tile.TileContext(nc) as tc, ExitStack() as ctx:
        pers = ctx.enter_context(tc.tile_pool(name="pers", bufs=1))
        loadp = ctx.enter_context(tc.tile_pool(name="load", bufs=4))
        tmpp = ctx.enter_context(tc.tile_pool(name="tmp", bufs=1))

        R = pers.tile([KP, L], F32, tag="R")
        R2 = pers.tile([KP, L], F32, tag="R2")
        etaR = pers.tile([KP, L], F32, tag="etaR")
        etaR2 = pers.tile([KP, L], F32, tag="etaR2")
        Aloc = pers.tile([KP, LE], F32, tag="Aloc")
        Bloc = pers.tile([KP, LE], F32, tag="Bloc")
        Aprev = pers.tile([KP, L], F32, tag="Aprev")
        Bprev = pers.tile([KP, L], F32, tag="Bprev")
        cvec = pers.tile([KP, KC], F32, tag="cvec")
        geoc = pers.tile([KP, L], F32, tag="geoc")
        initA = pers.tile([KP, 1], F32, tag="initA")
        initB = pers.tile([KP, 1], F32, tag="initB")
        qsum = pers.tile([KP, 1], F32, tag="qsum")
        qrow = pers.tile([1, KP - 1], F32, tag="qrow")
        qtot = pers.tile([1, 1], F32, tag="qtot")

        # constants / scan seeds
        nc.vector.memset(qtot[0:1, 0:1], 1.0)
        nc.scalar.sqrt(qtot[0:1, 0:1], qtot[0:1, 0:1])  # pin ACT table early
        nc.vector.memset(cvec[:, :], float(CDEC))
        nc.vector.memset(Aloc[:, 0:1], 0.0)
        nc.vector.memset(Bloc[:, 0:1], 0.0)
        nc.vector.memset(initA[0:1, 0:1], 0.0)
        nc.vector.memset(initB[0:1, 0:1], 0.0)
        nc.sync.dma_start(geoc[:], geoc_dram.ap())

        # ---- stage A: chunked bulk + scan extension ----
        for k in range(CH):
            ks = slice(k * KC, (k + 1) * KC)
            wt = loadp.tile([KP, FW], F32, tag="wt")
            rt = loadp.tile([KP, FW], F32, tag="rt")
            nc.sync.dma_start(wt[:], w_v[:, k * FW:(k + 1) * FW])
            nc.scalar.dma_start(rt[:], nr_v[:, k * FW:(k + 1) * FW])
            eng = nc.vector if k == CH - 1 else nc.gpsimd
            eng.tensor_mul(wt[:], wt[:], rt[:])
            nc.vector.reduce_sum(
                R[:, ks], wt[:].rearrange("p (t a) -> p t a", a=NA), axis=AX.X
            )
            # derived streams on ACT
            nc.scalar.mul(etaR[:, ks], R[:, ks], ETA)
            nc.scalar.activation(etaR2[:, ks], R[:, ks], AF.Square, scale=0.1)
            nc.scalar.square(R2[:, ks], R[:, ks])
            # chained zero-carry local scans: state = c*state + eta*x
            nc.vector.tensor_tensor_scan(
                out=Aloc[:, 1 + k * KC:1 + (k + 1) * KC], data0=cvec[:, :],
                data1=etaR[:, ks], initial=Aloc[:, k * KC:k * KC + 1],
                op0=AL.mult, op1=AL.add,
            )
            nc.vector.tensor_tensor_scan(
                out=Bloc[:, 1 + k * KC:1 + (k + 1) * KC], data0=cvec[:, :],
                data1=etaR2[:, ks], initial=Bloc[:, k * KC:k * KC + 1],
                op0=AL.mult, op1=AL.add,
            )

        # ---- tail ----
        # carries: previous partition's local final
        nc.sync.dma_start(initA[1:KP, 0:1], Aloc[0:KP - 1, L:LE])
        nc.scalar.dma_start(initB[1:KP, 0:1], Bloc[0:KP - 1, L:LE])

        # A_prev[:,t] = Aloc[:,t-1] + initA*c^t  (Aloc col0 is the zero pad)
        nc.vector.scalar_tensor_tensor(
            out=Aprev[:, :], in0=geoc[:, :], scalar=initA[:, 0:1],
            in1=Aloc[:, 0:L], op0=AL.mult, op1=AL.add,
        )
        nc.vector.scalar_tensor_tensor(
            out=Bprev[:, :], in0=geoc[:, :], scalar=initB[:, 0:1],
            in1=Bloc[:, 0:L], op0=AL.mult, op1=AL.add,
        )

        # ---- D chain: q = [0.5*A*(R^2+B) - B*R] / var^1.5 ----
        T1 = tmpp.tile([KP, L], F32, tag="T1")
        T2 = tmpp.tile([KP, L], F32, tag="T2")
        T3 = tmpp.tile([KP, L], F32, tag="T3")
        T4 = tmpp.tile([KP, L], F32, tag="T4")
        T5 = tmpp.tile([KP, L], F32, tag="T5")
        T6 = tmpp.tile([KP, L], F32, tag="T6")

        nc.gpsimd.tensor_add(T1[:, :], R2[:, :], Bprev[:, :])      # g1 = R^2+B
        nc.gpsimd.tensor_mul(T2[:, :], Bprev[:, :], R[:, :])       # g3 = B*R
        nc.vector.scalar_tensor_tensor(                            # g2 = 0.5A*g1
            out=T3[:, :], in0=Aprev[:, :], scalar=0.5, in1=T1[:, :],
            op0=AL.mult, op1=AL.mult,
        )
        nc.vector.tensor_sub(T3[:, :], T3[:, :], T2[:, :])         # negn = g2-g3
        nc.scalar.square(T4[:, :], Aprev[:, :])                    # a2 = A^2
        nc.vector.tensor_sub(T5[:, :], Bprev[:, :], T4[:, :])      # v = B-a2
        nc.vector.tensor_scalar_max(T5[:, :], T5[:, :], EPS)       # var
        nc.scalar.sqrt(T6[:, :], T5[:, :])                         # s = sqrt(var)
        nc.vector.tensor_mul(T4[:, :], T5[:, :], T6[:, :])         # d = var^1.5
        nc.vector.reciprocal_approx_accurate(T5[:, :], T4[:, :], T6[:, :])  # rec

        nc.vector.scalar_tensor_tensor(                            # qsum=sum(negn*rec)
            out=T4[:, :], in0=T3[:, :], scalar=1.0, in1=T5[:, :],
            op0=AL.mult, op1=AL.mult, accum_out=qsum[:, 0:1],
        )
        # partition reduce: flatten 125 partials to one row, reduce, store
        nc.sync.dma_start(qrow[0:1, 0:KP - 1], qsum[1:KP, 0:1])
        nc.vector.reduce_sum(qtot[0:1, 0:1], qrow[0:1, 0:KP - 1], axis=AX.X)
        nc.sync.dma_start(out_ap[0:1, 0:1], qtot[0:1, 0:1])

    nc.compile()
    return nc


def _get_program():
    global _PROGRAM
    if _PROGRAM is None:
        _PROGRAM = _build_program()
    return _PROGRAM


def _core0_prepend():
    """2000 synthetic rows encoding the global init (A,B)=(0,EPS).

    All-zero rows leave the scan at (0,0); the last two rows carry returns
    r1, r2 with r2 = -fl(c*r1) so the A-scan cancels to ~0, while
    eta*(c*r1^2 + r2^2) ~ EPS supplies the B carry.
    """
    w = np.zeros((L, NA), NF32)
    nr = np.zeros((L, NA), NF32)
    c = CDEC
    r1 = NF32(np.sqrt(EPS / (ETA * (float(c) + float(c) ** 2))))
    r2 = NF32(-(c * r1))
    w[L - 2, 0] = NF32(1.0)
    nr[L - 2, 0] = r1
    w[L - 1, 0] = NF32(1.0)
    nr[L - 1, 0] = r2
    return w, nr


def _make_in_maps(weights, nr):
    weights = np.ascontiguousarray(weights, dtype=NF32)
    nr = np.ascontiguousarray(nr, dtype=NF32)
    pre_w, pre_nr = _core0_prepend()
    in_maps = []
    for m in range(N_CORES):
        s = m * OWN
        if m == 0:
            wm = np.concatenate([pre_w, weights[:OWN]])
            rm = np.concatenate([pre_nr, nr[:OWN]])
        else:
            wm = weights[s - L:s + OWN]
            rm = nr[s - L:s + OWN]
        in_maps.append({"w": wm, "nr": rm})
    return in_maps


def _run(in_maps, **kwargs):
    nc = _get_program()
    return run_bass_kernel_spmd(nc, in_maps, core_ids=list(range(N_CORES)), **kwargs)


def kernel(weights, next_returns):
    in_maps = _make_in_maps(weights, next_returns)
    res = _run(in_maps)
    total = np.sum(
        np.array([res.results[m]["out"][0, 0] for m in range(N_CORES)], NF32),
        dtype=NF32,
    )
    return NF32(NF32(ETA) * total / NF32(B_TOTAL))



# revision 2
# speedup vs baseline: 1.2594x; 1.2594x over previous
"""Trainium2 Bass kernel for nn_DSRLossStateless (DSR loss, stateless).

loss = -sum_t(D_t)/B where D_t comes from an eta-EMA pair (A,B) over
portfolio returns R_t = sum_a w[t,a]*nr[t,a].

Strategy (8 cores, batch-sharded, fp16 bulk + scaled EMAs):
  - Each core owns 250k consecutive rows as 125 SBUF partitions x 2000
    cols (time-major); partition 0 holds the 2000 preceding rows
    (synthetic for core 0) and supplies the scan carry for partition 1.
  - Loads are SWDGE cast-DMAs (f32 HBM -> fp16 SBUF), 8 chunks per
    tensor. DVE does the w*nr product and a 16->1 pairwise add tree in
    fp16 2x mode; the last add emits f32 R.
  - Scaled EMAs keep everything fp16-friendly: A'' = 0.5*A comes from
    scanning 5e-3*R; B' = 1e4*B from scanning (10R)^2. Then
    negn4 = B'*(A''-R) + A''*(100R)^2   (= 1e4 * [0.5A(R^2+B) - BR])
    var4 = B' - (200*A'')^2             (= 1e4 * var)
    q = negn4/var4^1.5 = 1e-2 * negn/var^1.5, and with eta*100 = 1 the
    host loss is just sum(q)/B.
  - Scans are f32, chained per chunk; the partition carry is the
    previous partition's final (c^2000 ~ 1.9e-9 kills older terms),
    applied as A_prev = Aloc_shift + init*c^t in one STT -> fp16.
  - D-chain is fp16 TT ops; the reciprocal runs on f32 via
    reciprocal_approx_fast. Per-partition sums accumulate in f32.
"""

import sys

sys.path.insert(0, "/opt/trn_rl_repo")

import numpy as np

import concourse.bass as bass
import concourse.bacc as bacc
import concourse.tile as tile
from concourse import mybir
from concourse.bass_utils import run_bass_kernel_spmd
from contextlib import ExitStack

F32 = mybir.dt.float32
F16 = mybir.dt.float16
NF32 = np.float32

N_CORES = 8
NA = 16            # assets (inner dim)
KP = 126           # SBUF partitions used (0 = prepend/carry-feeder)
L = 2000           # columns (time steps per partition)
LE = L + 1         # local-scan buffer width (col 0 = zero carry)
OWN = (KP - 1) * L      # rows owned per core = 250000
B_TOTAL = N_CORES * OWN # 2000000
CH = 8             # stage-A chunks
KC = L // CH       # 250 time steps per partition per chunk
FW = KC * NA       # 4000 elems per partition per chunk tile
ETA = 0.01
EPS = 1e-8
CDEC = NF32(1.0 - ETA)  # 0.99

AL = mybir.AluOpType
AF = mybir.ActivationFunctionType
AX = mybir.AxisListType

_PROGRAM = None


def _build_program():
    nc = bacc.Bacc("TRN2", target_bir_lowering=False, debug=False)

    w_ap = nc.dram_tensor("w", [KP * L, NA], F32, kind="ExternalInput").ap()
    nr_ap = nc.dram_tensor("nr", [KP * L, NA], F32, kind="ExternalInput").ap()
    out_ap = nc.dram_tensor("out", [1, 1], F32, kind="ExternalOutput").ap()

    # geo[t] = c^t (carry decay for the correction pass)
    geo_np = (CDEC ** np.arange(L).astype(NF32)).astype(NF32)
    geo_dram = nc.inline_tensor(
        np.ascontiguousarray(np.broadcast_to(geo_np, (KP, L))), name="geoc"
    )

    w_v = w_ap.rearrange("(p t) a -> p (t a)", p=KP)
    nr_v = nr_ap.rearrange("(p t) a -> p (t a)", p=KP)

    with tile.TileContext(nc) as tc, ExitStack() as ctx:
        pers = ctx.enter_context(tc.tile_pool(name="pers", bufs=1))
        loadp = ctx.enter_context(tc.tile_pool(name="load", bufs=4))
        tmpp = ctx.enter_context(tc.tile_pool(name="tmp", bufs=2))

        sc5 = pers.tile([KP, L], F32, tag="sc5")     # 5e-3 * R (A''-scan in)
        sq10 = pers.tile([KP, L], F32, tag="sq10")   # (10R)^2  (B'-scan in)
        R2h = pers.tile([KP, L], F16, tag="R2h")     # (100R)^2
        R16 = pers.tile([KP, L], F16, tag="R16")     # R
        Aloc = pers.tile([KP, LE], F32, tag="Aloc")
        Bloc = pers.tile([KP, LE], F32, tag="Bloc")
        Ap16 = pers.tile([KP, L], F16, tag="Ap16")   # A'' prev (carried)
        Bp16 = pers.tile([KP, L], F16, tag="Bp16")   # B'  prev (carried)
        cvec = pers.tile([KP, KC], F32, tag="cvec")
        geoc = pers.tile([KP, L], F32, tag="geoc")
        initAB = pers.tile([KP, 2], F32, tag="initAB")
        t16a = pers.tile([KP, L], F16, tag="t16a")
        t16b = pers.tile([KP, L], F16, tag="t16b")
        t16c = pers.tile([KP, L], F16, tag="t16c")
        tf32a = pers.tile([KP, L], F32, tag="tf32a")
        tf32b = pers.tile([KP, L], F32, tag="tf32b")
        qsum = pers.tile([KP, 1], F32, tag="qsum")
        qrow = pers.tile([1, KP - 1], F32, tag="qrow")
        qtot = pers.tile([1, 1], F32, tag="qtot")

        # constants / scan seeds
        nc.vector.memset(qtot[0:1, 0:1], 1.0)
        nc.scalar.sqrt(qtot[0:1, 0:1], qtot[0:1, 0:1])  # pin ACT table early
        nc.vector.memset(cvec[:, :], float(CDEC))
        nc.vector.memset(Aloc[:, 0:1], 0.0)
        nc.vector.memset(Bloc[:, 0:1], 0.0)
        nc.vector.memset(initAB[:, :], 0.0)
        nc.sync.dma_start(geoc[:], geo_dram.ap())

        # ---- stage A: chunked cast-loads, product, 16->1 tree, scans ----
        for k in range(CH):
            ks = slice(k * KC, (k + 1) * KC)
            wt = loadp.tile([KP, FW], F16, tag="wt")
            rt = loadp.tile([KP, FW], F16, tag="rt")
            nc.gpsimd.dma_start(wt[:], w_v[:, k * FW:(k + 1) * FW])
            nc.gpsimd.dma_start(rt[:], nr_v[:, k * FW:(k + 1) * FW])

            prod = tmpp.tile([KP, FW], F16, tag="prod")
            s1 = tmpp.tile([KP, KC * 8], F16, tag="s1")
            s2 = tmpp.tile([KP, KC * 4], F16, tag="s2")
            s3 = tmpp.tile([KP, KC * 2], F16, tag="s3")
            nc.vector.tensor_mul(prod[:], wt[:], rt[:])
            p3 = prod[:].rearrange("p (t a) -> p t a", a=NA)
            s1v = s1[:].rearrange("p (t a) -> p t a", a=8)
            s2v = s2[:].rearrange("p (t a) -> p t a", a=4)
            s3v = s3[:].rearrange("p (t a) -> p t a", a=2)
            nc.vector.tensor_add(s1v[:, :, :], p3[:, :, 0:8], p3[:, :, 8:16])
            nc.vector.tensor_add(s2v[:, :, :], s1v[:, :, 0:4], s1v[:, :, 4:8])
            nc.vector.tensor_add(s3v[:, :, :], s2v[:, :, 0:2], s2v[:, :, 2:4])
            # last tree stage emits f32 directly into the scan input
            nc.vector.tensor_add(tf32a[:, ks], s3v[:, :, 0], s3v[:, :, 1])
            # derived streams on ACT (R lives in tf32a[:, ks])
            nc.scalar.mul(sc5[:, ks], tf32a[:, ks], 5e-3)
            nc.scalar.activation(sq10[:, ks], tf32a[:, ks], AF.Square, scale=10.0)
            nc.scalar.activation(R2h[:, ks], tf32a[:, ks], AF.Square, scale=100.0)
            nc.scalar.activation(R16[:, ks], tf32a[:, ks], AF.Copy)
            # chained scans: state = c*state + x
            nc.vector.tensor_tensor_scan(
                out=Aloc[:, 1 + k * KC:1 + (k + 1) * KC], data0=cvec[:, :],
                data1=sc5[:, ks], initial=Aloc[:, k * KC:k * KC + 1],
                op0=AL.mult, op1=AL.add,
            )
            nc.vector.tensor_tensor_scan(
                out=Bloc[:, 1 + k * KC:1 + (k + 1) * KC], data0=cvec[:, :],
                data1=sq10[:, ks], initial=Bloc[:, k * KC:k * KC + 1],
                op0=AL.mult, op1=AL.add,
            )

        # ---- tail ----
        # carries: previous partition's local final (c^2000 kills older terms)
        nc.sync.dma_start(initAB[1:KP, 0:1], Aloc[0:KP - 1, L:LE])
        nc.scalar.dma_start(initAB[1:KP, 1:2], Bloc[0:KP - 1, L:LE])

        # A''_prev[:,t] = Aloc[:,t] + init*c^t  (Aloc col0 is the zero pad)
        nc.vector.scalar_tensor_tensor(
            out=Ap16[:, :], in0=geoc[:, :], scalar=initAB[:, 0:1],
            in1=Aloc[:, 0:L], op0=AL.mult, op1=AL.add,
        )
        nc.vector.scalar_tensor_tensor(
            out=Bp16[:, :], in0=geoc[:, :], scalar=initAB[:, 1:2],
            in1=Bloc[:, 0:L], op0=AL.mult, op1=AL.add,
        )

        # ---- D chain (fp16): q = negn4 / var4^1.5 ----
        nc.vector.tensor_sub(t16a[:, :], Ap16[:, :], R16[:, :])    # u = A''-R
        nc.vector.tensor_mul(t16b[:, :], Bp16[:, :], t16a[:, :])   # m1 = B'*u
        nc.vector.tensor_mul(t16a[:, :], Ap16[:, :], R2h[:, :])    # m2 = A''*(100R)^2
        nc.vector.tensor_add(t16c[:, :], t16b[:, :], t16a[:, :])   # negn4
        nc.scalar.activation(t16a[:, :], Ap16[:, :], AF.Square, scale=200.0)  # (200A'')^2
        nc.vector.tensor_sub(t16b[:, :], Bp16[:, :], t16a[:, :])   # var4
        nc.scalar.sqrt(t16a[:, :], t16b[:, :])                     # s = sqrt(var4)
        nc.vector.tensor_mul(tf32a[:, :], t16b[:, :], t16a[:, :])  # d = var4^1.5 (f32)
        nc.vector.reciprocal_approx_fast(tf32b[:, :], tf32a[:, :])
        nc.vector.scalar_tensor_tensor(                            # qsum += negn4*rec
            out=tf32a[:, :], in0=t16c[:, :], scalar=1.0, in1=tf32b[:, :],
            op0=AL.mult, op1=AL.mult, accum_out=qsum[:, 0:1],
        )
        # partition reduce: flatten 125 partials to one row, reduce, store
        nc.sync.dma_start(qrow[0:1, 0:KP - 1], qsum[1:KP, 0:1])
        nc.vector.reduce_sum(qtot[0:1, 0:1], qrow[0:1, 0:KP - 1], axis=AX.X)
        nc.sync.dma_start(out_ap[0:1, 0:1], qtot[0:1, 0:1])

    nc.compile()
    return nc


def _get_program():
    global _PROGRAM
    if _PROGRAM is None:
        _PROGRAM = _build_program()
    return _PROGRAM


def _core0_prepend():
    """2000 synthetic rows encoding the global init (A,B)=(0,EPS).

    All-zero rows leave the scan at (0,0); the last two rows carry returns
    r1, r2 with r2 = -fl(c*r1) so the A-scan cancels to ~0, while
    eta*(c*r1^2 + r2^2) ~ EPS supplies the B carry.
    """
    w = np.zeros((L, NA), NF32)
    nr = np.zeros((L, NA), NF32)
    c = CDEC
    r1 = NF32(np.sqrt(EPS / (ETA * (float(c) + float(c) ** 2))))
    r2 = NF32(-(c * r1))
    w[L - 2, 0] = NF32(1.0)
    nr[L - 2, 0] = r1
    w[L - 1, 0] = NF32(1.0)
    nr[L - 1, 0] = r2
    return w, nr


def _make_in_maps(weights, nr):
    weights = np.ascontiguousarray(weights, dtype=NF32)
    nr = np.ascontiguousarray(nr, dtype=NF32)
    pre_w, pre_nr = _core0_prepend()
    in_maps = []
    for m in range(N_CORES):
        s = m * OWN
        if m == 0:
            wm = np.concatenate([pre_w, weights[:OWN]])
            rm = np.concatenate([pre_nr, nr[:OWN]])
        else:
            wm = weights[s - L:s + OWN]
            rm = nr[s - L:s + OWN]
        in_maps.append({"w": wm, "nr": rm})
    return in_maps


def _run(in_maps, **kwargs):
    nc = _get_program()
    return run_bass_kernel_spmd(nc, in_maps, core_ids=list(range(N_CORES)), **kwargs)


def kernel(weights, next_returns):
    in_maps = _make_in_maps(weights, next_returns)
    res = _run(in_maps)
    total = np.sum(
        np.array([res.results[m]["out"][0, 0] for m in range(N_CORES)], NF32),
        dtype=NF32,
    )
    # q = 1e-2 * negn/var^1.5 and loss = eta*sum(negn/var^1.5)/B, eta*100 = 1
    return NF32(total / NF32(B_TOTAL))


# revision 3
# speedup vs baseline: 1.3209x; 1.0488x over previous
"""Trainium2 Bass kernel for nn_DSRLossStateless (DSR loss, stateless).

loss = -sum_t(D_t)/B where D_t comes from an eta-EMA pair (A,B) over
portfolio returns R_t = sum_a w[t,a]*nr[t,a].

Strategy (8 cores, batch-sharded, fp16 bulk + scaled EMAs):
  - Each core owns 250k consecutive rows as 125 SBUF partitions x 2000
    cols (time-major); partition 0 holds the 2000 preceding rows
    (synthetic for core 0) and supplies the scan carry for partition 1.
  - Loads are SWDGE cast-DMAs (f32 HBM -> fp16 SBUF) issued up front;
    chunk columns taper at the end ([250x7,150,100]) so the serial
    after-last-DMA work is small. DVE does the w*nr product and a
    16->1 pairwise add tree in fp16 2x mode; the last add emits f32 R.
  - Scaled EMAs keep everything fp16-friendly: A'' = 0.5*A comes from
    scanning 5e-3*R; B' = 1e4*B from scanning (10R)^2. Then
    negn4 = B'*(A''-R) + A''*(100R)^2   (= 1e4 * [0.5A(R^2+B) - BR])
    var4 = B' - (200*A'')^2             (= 1e4 * var)
    q = negn4/var4^1.5 = 1e-2 * negn/var^1.5, and with eta*100 = 1 the
    host loss is just sum(q)/B.
  - Scans are f32, chained per chunk; the partition carry is the
    previous partition's final (c^2000 ~ 1.9e-9 kills older terms),
    applied as A_prev = Aloc_shift + init*c^t in one STT -> fp16.
  - The D-chain is fp16 TT ops split into two column halves with
    interleaved issue so ACT square/sqrt hide under DVE work; the
    reciprocal runs on f32 via reciprocal_approx_fast. Per-partition
    sums land in qsum[:,0:2] and ship to the host, which reduces.
"""

import sys

sys.path.insert(0, "/opt/trn_rl_repo")

import numpy as np

import concourse.bass as bass
import concourse.bacc as bacc
import concourse.tile as tile
from concourse import mybir
from concourse.bass_utils import run_bass_kernel_spmd
from contextlib import ExitStack

F32 = mybir.dt.float32
F16 = mybir.dt.float16
NF32 = np.float32

N_CORES = 8
NA = 16            # assets (inner dim)
KP = 126           # SBUF partitions used (0 = prepend/carry-feeder)
L = 2000           # columns (time steps per partition)
LE = L + 1         # local-scan buffer width (col 0 = zero carry)
OWN = (KP - 1) * L      # rows owned per core = 250000
B_TOTAL = N_CORES * OWN # 2000000
CHUNKS = [250] * 7 + [150, 100]   # column count per stage-A chunk
KCMAX = max(CHUNKS)
ETA = 0.01
EPS = 1e-8
CDEC = NF32(1.0 - ETA)  # 0.99
HL = L // 2        # tail half width

AL = mybir.AluOpType
AF = mybir.ActivationFunctionType
AX = mybir.AxisListType

_PROGRAM = None


def _build_program():
    nc = bacc.Bacc("TRN2", target_bir_lowering=False, debug=False)

    w_ap = nc.dram_tensor("w", [KP * L, NA], F32, kind="ExternalInput").ap()
    nr_ap = nc.dram_tensor("nr", [KP * L, NA], F32, kind="ExternalInput").ap()
    out_ap = nc.dram_tensor("out", [KP, 2], F32, kind="ExternalOutput").ap()

    # geo[t] = c^t (carry decay for the correction pass), fp16 is plenty:
    # where c^t underflows fp16 the carry term is ~1e-9 of Aloc anyway.
    geo_np = (CDEC ** np.arange(L).astype(NF32)).astype(np.float16)
    geo_dram = nc.inline_tensor(
        np.ascontiguousarray(np.broadcast_to(geo_np, (KP, L))), name="geoc"
    )

    w_v = w_ap.rearrange("(p t) a -> p (t a)", p=KP)
    nr_v = nr_ap.rearrange("(p t) a -> p (t a)", p=KP)

    with tile.TileContext(nc) as tc, ExitStack() as ctx:
        pers = ctx.enter_context(tc.tile_pool(name="pers", bufs=1))
        loadp = ctx.enter_context(tc.tile_pool(name="load", bufs=4))
        tmpp = ctx.enter_context(tc.tile_pool(name="tmp", bufs=2))

        sc5 = pers.tile([KP, L], F32, tag="sc5")     # 5e-3 * R (A''-scan in)
        sq10 = pers.tile([KP, L], F32, tag="sq10")   # (10R)^2  (B'-scan in)
        R2h = pers.tile([KP, L], F16, tag="R2h")     # (100R)^2
        R16 = pers.tile([KP, L], F16, tag="R16")     # R
        Aloc = pers.tile([KP, LE], F32, tag="Aloc")
        Bloc = pers.tile([KP, LE], F32, tag="Bloc")
        Ap16 = pers.tile([KP, L], F16, tag="Ap16")   # A'' prev (carried)
        Bp16 = pers.tile([KP, L], F16, tag="Bp16")   # B'  prev (carried)
        cvec = pers.tile([KP, KCMAX], F32, tag="cvec")
        geoc = pers.tile([KP, L], F16, tag="geoc")
        initAB = pers.tile([KP, 2], F32, tag="initAB")
        t16a = pers.tile([KP, L], F16, tag="t16a")
        t16b = pers.tile([KP, L], F16, tag="t16b")
        t16c = pers.tile([KP, L], F16, tag="t16c")
        tf32a = pers.tile([KP, L], F32, tag="tf32a")
        tf32b = pers.tile([KP, L], F32, tag="tf32b")
        qsum = pers.tile([KP, 2], F32, tag="qsum")

        # chunk loads first: the SDMA stream is the critical resource
        tiles = []
        off = 0
        for k, kc in enumerate(CHUNKS):
            fw = kc * NA
            wt = loadp.tile([KP, KCMAX * NA], F16, tag="wt")
            rt = loadp.tile([KP, KCMAX * NA], F16, tag="rt")
            nc.gpsimd.dma_start(wt[:, 0:fw], w_v[:, off * NA:(off + kc) * NA])
            nc.gpsimd.dma_start(rt[:, 0:fw], nr_v[:, off * NA:(off + kc) * NA])
            tiles.append((off, kc, wt, rt))
            off += kc

        # constants / scan seeds (DVE/ACT; do not block the DMA stream)
        nc.vector.memset(qsum[0:1, 0:1], 1.0)
        nc.scalar.sqrt(qsum[0:1, 0:1], qsum[0:1, 0:1])  # pin ACT table early
        nc.vector.memset(cvec[:, :], float(CDEC))
        nc.vector.memset(Aloc[:, 0:1], 0.0)
        nc.vector.memset(Bloc[:, 0:1], 0.0)
        nc.vector.memset(initAB[:, :], 0.0)
        nc.scalar.dma_start(geoc[:], geo_dram.ap())

        # ---- stage A: product, 16->1 tree, derived streams, scans ----
        for off, kc, wt, rt in tiles:
            fw = kc * NA
            ks = slice(off, off + kc)
            prod = tmpp.tile([KP, KCMAX * NA], F16, tag="prod")
            s1 = tmpp.tile([KP, KCMAX * 8], F16, tag="s1")
            s2 = tmpp.tile([KP, KCMAX * 4], F16, tag="s2")
            s3 = tmpp.tile([KP, KCMAX * 2], F16, tag="s3")
            nc.vector.tensor_mul(prod[:, 0:fw], wt[:, 0:fw], rt[:, 0:fw])
            p3 = prod[:, 0:fw].rearrange("p (t a) -> p t a", a=NA)
            s1v = s1[:, 0:kc * 8].rearrange("p (t a) -> p t a", a=8)
            s2v = s2[:, 0:kc * 4].rearrange("p (t a) -> p t a", a=4)
            s3v = s3[:, 0:kc * 2].rearrange("p (t a) -> p t a", a=2)
            nc.vector.tensor_add(s1v[:, :, :], p3[:, :, 0:8], p3[:, :, 8:16])
            nc.vector.tensor_add(s2v[:, :, :], s1v[:, :, 0:4], s1v[:, :, 4:8])
            nc.vector.tensor_add(s3v[:, :, :], s2v[:, :, 0:2], s2v[:, :, 2:4])
            # last tree stage emits f32 R
            nc.vector.tensor_add(tf32a[:, ks], s3v[:, :, 0], s3v[:, :, 1])
            # derived streams on ACT
            nc.scalar.mul(sc5[:, ks], tf32a[:, ks], 5e-3)
            nc.scalar.activation(sq10[:, ks], tf32a[:, ks], AF.Square, scale=10.0)
            nc.scalar.activation(R2h[:, ks], tf32a[:, ks], AF.Square, scale=100.0)
            nc.scalar.activation(R16[:, ks], tf32a[:, ks], AF.Copy)
            # chained scans: state = c*state + x
            nc.vector.tensor_tensor_scan(
                out=Aloc[:, 1 + off:1 + off + kc], data0=cvec[:, 0:kc],
                data1=sc5[:, ks], initial=Aloc[:, off:off + 1],
                op0=AL.mult, op1=AL.add,
            )
            nc.vector.tensor_tensor_scan(
                out=Bloc[:, 1 + off:1 + off + kc], data0=cvec[:, 0:kc],
                data1=sq10[:, ks], initial=Bloc[:, off:off + 1],
                op0=AL.mult, op1=AL.add,
            )

        # ---- tail ----
        # carries: previous partition's local final (c^2000 kills older terms)
        nc.sync.dma_start(initAB[1:KP, 0:1], Aloc[0:KP - 1, L:LE])
        nc.scalar.dma_start(initAB[1:KP, 1:2], Bloc[0:KP - 1, L:LE])

        def corr(h):
            # A''_prev[:,t] = Aloc[:,t] + init*c^t (Aloc col0 is the zero pad)
            nc.vector.scalar_tensor_tensor(
                out=Ap16[:, h], in0=geoc[:, h], scalar=initAB[:, 0:1],
                in1=Aloc[:, h], op0=AL.mult, op1=AL.add,
            )
            # ACT square can start for this half right away
            nc.scalar.activation(t16a[:, h], Ap16[:, h], AF.Square, scale=200.0)
            nc.vector.scalar_tensor_tensor(
                out=Bp16[:, h], in0=geoc[:, h], scalar=initAB[:, 1:2],
                in1=Bloc[:, h], op0=AL.mult, op1=AL.add,
            )

        def negn_chain(h):
            nc.vector.tensor_sub(t16b[:, h], Ap16[:, h], R16[:, h])   # u
            nc.vector.tensor_mul(t16c[:, h], Bp16[:, h], t16b[:, h])  # m1
            nc.vector.tensor_mul(t16b[:, h], Ap16[:, h], R2h[:, h])   # m2
            nc.vector.tensor_add(t16c[:, h], t16c[:, h], t16b[:, h])  # negn4
            nc.vector.tensor_sub(t16b[:, h], Bp16[:, h], t16a[:, h])  # var4
            nc.scalar.sqrt(t16a[:, h], t16b[:, h])                    # s

        def var_chain(h, col):
            nc.vector.tensor_mul(tf32a[:, h], t16b[:, h], t16a[:, h])  # var4^1.5
            nc.vector.reciprocal_approx_fast(tf32b[:, h], tf32a[:, h])
            nc.vector.scalar_tensor_tensor(                            # qsum+=negn*rec
                out=tf32a[:, h], in0=t16c[:, h], scalar=1.0, in1=tf32b[:, h],
                op0=AL.mult, op1=AL.mult, accum_out=qsum[:, col],
            )

        h0 = slice(0, HL)
        h1 = slice(HL, L)
        corr(h0)
        negn_chain(h0)
        corr(h1)
        var_chain(h0, slice(0, 1))
        negn_chain(h1)
        var_chain(h1, slice(1, 2))

        # per-partition partials ship out; the host reduces (p0 excluded)
        nc.sync.dma_start(out_ap[:, :], qsum[:, :])

    nc.compile()
    return nc


def _get_program():
    global _PROGRAM
    if _PROGRAM is None:
        _PROGRAM = _build_program()
    return _PROGRAM


def _core0_prepend():
    """2000 synthetic rows encoding the global init (A,B)=(0,EPS).

    All-zero rows leave the scan at (0,0); the last two rows carry returns
    r1, r2 with r2 = -fl(c*r1) so the A-scan cancels to ~0, while
    eta*(c*r1^2 + r2^2) ~ EPS supplies the B carry.
    """
    w = np.zeros((L, NA), NF32)
    nr = np.zeros((L, NA), NF32)
    c = CDEC
    r1 = NF32(np.sqrt(EPS / (ETA * (float(c) + float(c) ** 2))))
    r2 = NF32(-(c * r1))
    w[L - 2, 0] = NF32(1.0)
    nr[L - 2, 0] = r1
    w[L - 1, 0] = NF32(1.0)
    nr[L - 1, 0] = r2
    return w, nr


def _make_in_maps(weights, nr):
    weights = np.ascontiguousarray(weights, dtype=NF32)
    nr = np.ascontiguousarray(nr, dtype=NF32)
    pre_w, pre_nr = _core0_prepend()
    in_maps = []
    for m in range(N_CORES):
        s = m * OWN
        if m == 0:
            wm = np.concatenate([pre_w, weights[:OWN]])
            rm = np.concatenate([pre_nr, nr[:OWN]])
        else:
            wm = weights[s - L:s + OWN]
            rm = nr[s - L:s + OWN]
        in_maps.append({"w": wm, "nr": rm})
    return in_maps


def _run(in_maps, **kwargs):
    nc = _get_program()
    return run_bass_kernel_spmd(nc, in_maps, core_ids=list(range(N_CORES)), **kwargs)


def kernel(weights, next_returns):
    in_maps = _make_in_maps(weights, next_returns)
    res = _run(in_maps)
    total = NF32(0.0)
    for m in range(N_CORES):
        q = np.asarray(res.results[m]["out"], NF32)
        total = NF32(total + np.sum(q[1:, :], dtype=NF32))
    # q = 1e-2 * negn/var^1.5 and loss = eta*sum(negn/var^1.5)/B, eta*100 = 1
    return NF32(total / NF32(B_TOTAL))


# revision 8
# speedup vs baseline: 1.3266x; 1.0044x over previous
"""Trainium2 Bass kernel for nn_DSRLossStateless (DSR loss, stateless).

loss = -sum_t(D_t)/B where D_t comes from an eta-EMA pair (A,B) over
portfolio returns R_t = sum_a w[t,a]*nr[t,a].

Strategy (8 cores, batch-sharded, fp16 bulk + scaled EMAs):
  - Each core owns 250k consecutive rows as 125 SBUF partitions x 2000
    cols (time-major); partition 0 holds the 2000 preceding rows
    (synthetic for core 0) and supplies the scan carry for partition 1.
  - Loads are SWDGE cast-DMAs (f32 HBM -> fp16 SBUF) issued up front;
    chunk columns taper at the end ([250x7,150,100]) so the serial
    after-last-DMA work is small. DVE does the w*nr product and a
    16->1 pairwise add tree in fp16 2x mode; the last add emits f32 R.
  - Scaled EMAs keep everything fp16-friendly: A'' = 0.5*A comes from
    scanning 5e-3*R; B' = 1e4*B from scanning (10R)^2. Then
    negn4 = B'*(A''-R) + A''*(100R)^2   (= 1e4 * [0.5A(R^2+B) - BR])
    var4 = B' - (200*A'')^2             (= 1e4 * var)
    q = negn4/var4^1.5 = 1e-2 * negn/var^1.5, and with eta*100 = 1 the
    host loss is just sum(q)/B.
  - Scans are f32, chained per chunk; the partition carry is the
    previous partition's final (c^2000 ~ 1.9e-9 kills older terms),
    applied as A_prev = Aloc_shift + init*c^t in one STT -> fp16.
  - The D-chain is fp16 TT ops split into two column halves with
    interleaved issue so ACT square/sqrt hide under DVE work; the
    reciprocal runs on f32 via reciprocal_approx_fast. Per-partition
    sums land in qsum[:,0:2] and ship to the host, which reduces.
"""

import sys

sys.path.insert(0, "/opt/trn_rl_repo")

import numpy as np

import concourse.bass as bass
import concourse.bacc as bacc
import concourse.tile as tile
from concourse import mybir
from concourse.bass_utils import run_bass_kernel_spmd
from contextlib import ExitStack

F32 = mybir.dt.float32
F16 = mybir.dt.float16
NF32 = np.float32

N_CORES = 8
NA = 16            # assets (inner dim)
KP = 126           # SBUF partitions used (0 = prepend/carry-feeder)
L = 2000           # columns (time steps per partition)
LE = L + 1         # local-scan buffer width (col 0 = zero carry)
OWN = (KP - 1) * L      # rows owned per core = 250000
B_TOTAL = N_CORES * OWN # 2000000
CHUNKS = [250] * 7 + [150, 100]   # column count per stage-A chunk
KCMAX = max(CHUNKS)
ETA = 0.01
EPS = 1e-8
CDEC = NF32(1.0 - ETA)  # 0.99
HL = L // 2        # tail half width

AL = mybir.AluOpType
AF = mybir.ActivationFunctionType
AX = mybir.AxisListType

_PROGRAM = None


def _build_program():
    nc = bacc.Bacc("TRN2", target_bir_lowering=False, debug=False)

    w_ap = nc.dram_tensor("w", [KP * L, NA], F32, kind="ExternalInput").ap()
    nr_ap = nc.dram_tensor("nr", [KP * L, NA], F32, kind="ExternalInput").ap()
    out_ap = nc.dram_tensor("out", [KP, 8], F32, kind="ExternalOutput").ap()

    # geo[t] = c^t (carry decay for the correction pass), fp16 is plenty:
    # where c^t underflows fp16 the carry term is ~1e-9 of Aloc anyway.
    geo_np = (CDEC ** np.arange(L).astype(NF32)).astype(np.float16)
    geo_dram = nc.inline_tensor(
        np.ascontiguousarray(np.broadcast_to(geo_np, (KP, L))), name="geoc"
    )

    w_v = w_ap.rearrange("(p t) a -> p (t a)", p=KP)
    nr_v = nr_ap.rearrange("(p t) a -> p (t a)", p=KP)

    with tile.TileContext(nc) as tc, ExitStack() as ctx:
        pers = ctx.enter_context(tc.tile_pool(name="pers", bufs=1))
        loadp = ctx.enter_context(tc.tile_pool(name="load", bufs=4))
        tmpp = ctx.enter_context(tc.tile_pool(name="tmp", bufs=2))

        sc5 = pers.tile([KP, L], F32, tag="sc5")     # 5e-3 * R (A''-scan in)
        sq10 = pers.tile([KP, L], F32, tag="sq10")   # (10R)^2  (B'-scan in)
        R2h = pers.tile([KP, L], F16, tag="R2h")     # (100R)^2
        R16 = pers.tile([KP, L], F16, tag="R16")     # R
        Aloc = pers.tile([KP, LE], F32, tag="Aloc")
        Bloc = pers.tile([KP, LE], F32, tag="Bloc")
        Ap16 = pers.tile([KP, L], F16, tag="Ap16")   # A'' prev (carried)
        Bp16 = pers.tile([KP, L], F16, tag="Bp16")   # B'  prev (carried)
        cvec = pers.tile([KP, KCMAX], F32, tag="cvec")
        geoc = pers.tile([KP, L], F16, tag="geoc")
        initAB = pers.tile([KP, 2], F32, tag="initAB")
        t16a = pers.tile([KP, L], F16, tag="t16a")
        t16b = pers.tile([KP, L], F16, tag="t16b")
        t16c = pers.tile([KP, L], F16, tag="t16c")
        tf32a = pers.tile([KP, L], F32, tag="tf32a")
        tf32b = pers.tile([KP, L], F32, tag="tf32b")
        qsum = pers.tile([KP, 8], F32, tag="qsum")

        # chunk loads first: the SDMA stream is the critical resource
        tiles = []
        off = 0
        for k, kc in enumerate(CHUNKS):
            fw = kc * NA
            wt = loadp.tile([KP, KCMAX * NA], F16, tag="wt")
            rt = loadp.tile([KP, KCMAX * NA], F16, tag="rt")
            nc.gpsimd.dma_start(wt[:, 0:fw], w_v[:, off * NA:(off + kc) * NA])
            nc.gpsimd.dma_start(rt[:, 0:fw], nr_v[:, off * NA:(off + kc) * NA])
            tiles.append((off, kc, wt, rt))
            off += kc

        # constants / scan seeds (DVE/ACT; do not block the DMA stream)
        nc.vector.memset(qsum[0:1, 0:1], 1.0)
        nc.scalar.sqrt(qsum[0:1, 0:1], qsum[0:1, 0:1])  # pin ACT table early
        nc.vector.memset(cvec[:, :], float(CDEC))
        nc.vector.memset(Aloc[:, 0:1], 0.0)
        nc.vector.memset(Bloc[:, 0:1], 0.0)
        nc.vector.memset(initAB[:, :], 0.0)
        nc.scalar.dma_start(geoc[:], geo_dram.ap())

        def negn_chain(h):
            nc.vector.tensor_sub(t16b[:, h], Ap16[:, h], R16[:, h])   # u
            nc.vector.tensor_mul(t16c[:, h], Bp16[:, h], t16b[:, h])  # m1
            nc.vector.tensor_mul(t16b[:, h], Ap16[:, h], R2h[:, h])   # m2
            nc.vector.tensor_add(t16c[:, h], t16c[:, h], t16b[:, h])  # negn4
            nc.vector.tensor_sub(t16b[:, h], Bp16[:, h], t16a[:, h])  # var4
            nc.scalar.sqrt(t16a[:, h], t16b[:, h])                    # s

        def var_chain(h, col):
            nc.vector.tensor_mul(tf32a[:, h], t16b[:, h], t16a[:, h])  # var4^1.5
            nc.vector.reciprocal_approx_fast(tf32b[:, h], tf32a[:, h])
            nc.vector.scalar_tensor_tensor(                            # qsum+=negn*rec
                out=tf32a[:, h], in0=t16c[:, h], scalar=1.0, in1=tf32b[:, h],
                op0=AL.mult, op1=AL.mult, accum_out=qsum[:, col],
            )

        # zero-carry D-chain for a chunk with off >= 1000: there the carry
        # term init*c^t is below fp16 resolution, so Aloc/Bloc are exact
        # enough and the chain can run inside stage-A DMA slack.
        def chain_nocarry(off, kc, col):
            h = slice(off, off + kc)
            nc.vector.tensor_copy(Ap16[:, h], Aloc[:, off:off + kc])
            nc.scalar.activation(t16a[:, h], Ap16[:, h], AF.Square, scale=200.0)
            nc.vector.tensor_copy(Bp16[:, h], Bloc[:, off:off + kc])
            negn_chain(h)
            var_chain(h, col)

        # ---- stage A: product, 16->1 tree, derived streams, scans ----
        qcol = 2
        for off, kc, wt, rt in tiles:
            fw = kc * NA
            ks = slice(off, off + kc)
            prod = tmpp.tile([KP, KCMAX * NA], F16, tag="prod")
            s1 = tmpp.tile([KP, KCMAX * 8], F16, tag="s1")
            s2 = tmpp.tile([KP, KCMAX * 4], F16, tag="s2")
            s3 = tmpp.tile([KP, KCMAX * 2], F16, tag="s3")
            nc.vector.tensor_mul(prod[:, 0:fw], wt[:, 0:fw], rt[:, 0:fw])
            p3 = prod[:, 0:fw].rearrange("p (t a) -> p t a", a=NA)
            s1v = s1[:, 0:kc * 8].rearrange("p (t a) -> p t a", a=8)
            s2v = s2[:, 0:kc * 4].rearrange("p (t a) -> p t a", a=4)
            s3v = s3[:, 0:kc * 2].rearrange("p (t a) -> p t a", a=2)
            nc.vector.tensor_add(s1v[:, :, :], p3[:, :, 0:8], p3[:, :, 8:16])
            nc.vector.tensor_add(s2v[:, :, :], s1v[:, :, 0:4], s1v[:, :, 4:8])
            nc.vector.tensor_add(s3v[:, :, :], s2v[:, :, 0:2], s2v[:, :, 2:4])
            # last tree stage emits f32 R
            nc.vector.tensor_add(tf32a[:, ks], s3v[:, :, 0], s3v[:, :, 1])
            # derived streams on ACT
            nc.scalar.mul(sc5[:, ks], tf32a[:, ks], 5e-3)
            nc.scalar.activation(sq10[:, ks], tf32a[:, ks], AF.Square, scale=10.0)
            nc.scalar.activation(R2h[:, ks], tf32a[:, ks], AF.Square, scale=100.0)
            nc.scalar.activation(R16[:, ks], tf32a[:, ks], AF.Copy)
            # chained scans: state = c*state + x
            nc.vector.tensor_tensor_scan(
                out=Aloc[:, 1 + off:1 + off + kc], data0=cvec[:, 0:kc],
                data1=sc5[:, ks], initial=Aloc[:, off:off + 1],
                op0=AL.mult, op1=AL.add,
            )
            nc.vector.tensor_tensor_scan(
                out=Bloc[:, 1 + off:1 + off + kc], data0=cvec[:, 0:kc],
                data1=sq10[:, ks], initial=Bloc[:, off:off + 1],
                op0=AL.mult, op1=AL.add,
            )
            if off >= HL:
                chain_nocarry(off, kc, slice(qcol, qcol + 1))
                qcol += 1

        # ---- tail ----
        # carries: previous partition's local final (c^2000 kills older terms)
        nc.sync.dma_start(initAB[1:KP, 0:1], Aloc[0:KP - 1, L:LE])
        nc.scalar.dma_start(initAB[1:KP, 1:2], Bloc[0:KP - 1, L:LE])

        def corr(h):
            # A''_prev[:,t] = Aloc[:,t] + init*c^t (Aloc col0 is the zero pad)
            nc.vector.scalar_tensor_tensor(
                out=Ap16[:, h], in0=geoc[:, h], scalar=initAB[:, 0:1],
                in1=Aloc[:, h], op0=AL.mult, op1=AL.add,
            )
            # ACT square can start for this half right away
            nc.scalar.activation(t16a[:, h], Ap16[:, h], AF.Square, scale=200.0)
            nc.vector.scalar_tensor_tensor(
                out=Bp16[:, h], in0=geoc[:, h], scalar=initAB[:, 1:2],
                in1=Bloc[:, h], op0=AL.mult, op1=AL.add,
            )

        # carried pass over the first half only (two quarters interleaved to
        # hide ACT under DVE)
        q0 = slice(0, HL // 2)
        q1 = slice(HL // 2, HL)
        corr(q0)
        negn_chain(q0)
        corr(q1)
        var_chain(q0, slice(0, 1))
        negn_chain(q1)
        var_chain(q1, slice(1, 2))

        # per-partition partials ship out; the host reduces (p0 excluded)
        nc.sync.dma_start(out_ap[:, :], qsum[:, :])

    nc.compile()
    return nc


def _get_program():
    global _PROGRAM
    if _PROGRAM is None:
        _PROGRAM = _build_program()
    return _PROGRAM


def _core0_prepend():
    """2000 synthetic rows encoding the global init (A,B)=(0,EPS).

    All-zero rows leave the scan at (0,0); the last two rows carry returns
    r1, r2 with r2 = -fl(c*r1) so the A-scan cancels to ~0, while
    eta*(c*r1^2 + r2^2) ~ EPS supplies the B carry.
    """
    w = np.zeros((L, NA), NF32)
    nr = np.zeros((L, NA), NF32)
    c = CDEC
    r1 = NF32(np.sqrt(EPS / (ETA * (float(c) + float(c) ** 2))))
    r2 = NF32(-(c * r1))
    w[L - 2, 0] = NF32(1.0)
    nr[L - 2, 0] = r1
    w[L - 1, 0] = NF32(1.0)
    nr[L - 1, 0] = r2
    return w, nr


def _make_in_maps(weights, nr):
    weights = np.ascontiguousarray(weights, dtype=NF32)
    nr = np.ascontiguousarray(nr, dtype=NF32)
    pre_w, pre_nr = _core0_prepend()
    in_maps = []
    for m in range(N_CORES):
        s = m * OWN
        if m == 0:
            wm = np.concatenate([pre_w, weights[:OWN]])
            rm = np.concatenate([pre_nr, nr[:OWN]])
        else:
            wm = weights[s - L:s + OWN]
            rm = nr[s - L:s + OWN]
        in_maps.append({"w": wm, "nr": rm})
    return in_maps


def _run(in_maps, **kwargs):
    nc = _get_program()
    return run_bass_kernel_spmd(nc, in_maps, core_ids=list(range(N_CORES)), **kwargs)


def kernel(weights, next_returns):
    in_maps = _make_in_maps(weights, next_returns)
    res = _run(in_maps)
    total = NF32(0.0)
    for m in range(N_CORES):
        q = np.asarray(res.results[m]["out"], NF32)
        total = NF32(total + np.sum(q[1:, :], dtype=NF32))
    # q = 1e-2 * negn/var^1.5 and loss = eta*sum(negn/var^1.5)/B, eta*100 = 1
    return NF32(total / NF32(B_TOTAL))


# revision 12
# speedup vs baseline: 1.3509x; 1.0183x over previous
"""Trainium2 Bass kernel for nn_DSRLossStateless (DSR loss, stateless).

loss = -sum_t(D_t)/B where D_t comes from an eta-EMA pair (A,B) over
portfolio returns R_t = sum_a w[t,a]*nr[t,a].

Strategy (8 cores, batch-sharded, fp16 bulk + scaled EMAs):
  - Each core owns 250k consecutive rows as 125 SBUF partitions x 2000
    cols (time-major); partition 0 holds the 2000 preceding rows
    (synthetic for core 0) and supplies the scan carry for partition 1.
  - Loads are SWDGE cast-DMAs (f32 HBM -> fp16 SBUF) issued up front;
    chunk columns taper at the end ([250x7,150,100]) so the serial
    after-last-DMA work is small. DVE does the w*nr product and a
    16->1 pairwise add tree in fp16 2x mode; the last add emits f32 R.
  - Scaled EMAs keep everything fp16-friendly: A'' = 0.5*A comes from
    scanning 5e-3*R; B' = 1e4*B from scanning (10R)^2. Then
    negn4 = B'*(A''-R) + A''*(100R)^2   (= 1e4 * [0.5A(R^2+B) - BR])
    var4 = B' - (200*A'')^2             (= 1e4 * var)
    q = negn4/var4^1.5 = 1e-2 * negn/var^1.5, and with eta*100 = 1 the
    host loss is just sum(q)/B.
  - Scans are f32, chained per chunk; the partition carry is the
    previous partition's final (c^2000 ~ 1.9e-9 kills older terms),
    applied as A_prev = Aloc_shift + init*c^t in one STT -> fp16.
  - The D-chain is fp16 TT ops split into two column halves with
    interleaved issue so ACT square/sqrt hide under DVE work; the
    reciprocal runs on f32 via reciprocal_approx_fast. Per-partition
    sums land in qsum[:,0:2] and ship to the host, which reduces.
"""

import sys

sys.path.insert(0, "/opt/trn_rl_repo")

import numpy as np

import concourse.bass as bass
import concourse.bacc as bacc
import concourse.tile as tile
from concourse import mybir
from concourse.bass_utils import run_bass_kernel_spmd
from contextlib import ExitStack

F32 = mybir.dt.float32
F16 = mybir.dt.float16
NF32 = np.float32

N_CORES = 8
NA = 16            # assets (inner dim)
KP = 126           # SBUF partitions used (0 = prepend/carry-feeder)
L = 2000           # columns (time steps per partition)
LE = L + 1         # local-scan buffer width (col 0 = zero carry)
OWN = (KP - 1) * L      # rows owned per core = 250000
B_TOTAL = N_CORES * OWN # 2000000
CHUNKS = [250] * 7 + [150, 100]   # column count per stage-A chunk
KCMAX = max(CHUNKS)
ETA = 0.01
EPS = 1e-8
CDEC = NF32(1.0 - ETA)  # 0.99
CUT = 750          # cols >= CUT use the zero-carry chain (init*c^t < fp16 ulp)

AL = mybir.AluOpType
AF = mybir.ActivationFunctionType
AX = mybir.AxisListType

_PROGRAM = None


def _build_program():
    nc = bacc.Bacc("TRN2", target_bir_lowering=False, debug=False)

    w_ap = nc.dram_tensor("w", [KP * L, NA], F32, kind="ExternalInput").ap()
    nr_ap = nc.dram_tensor("nr", [KP * L, NA], F32, kind="ExternalInput").ap()
    out_ap = nc.dram_tensor("out", [KP, 8], F32, kind="ExternalOutput").ap()

    # geo[t] = c^t (carry decay for the correction pass), fp16 is plenty:
    # where c^t underflows fp16 the carry term is ~1e-9 of Aloc anyway.
    geo_np = (CDEC ** np.arange(L).astype(NF32)).astype(np.float16)
    geo_dram = nc.inline_tensor(
        np.ascontiguousarray(np.broadcast_to(geo_np, (KP, L))), name="geoc"
    )

    w_v = w_ap.rearrange("(p t) a -> p (t a)", p=KP)
    nr_v = nr_ap.rearrange("(p t) a -> p (t a)", p=KP)

    with tile.TileContext(nc) as tc, ExitStack() as ctx:
        pers = ctx.enter_context(tc.tile_pool(name="pers", bufs=1))
        loadp = ctx.enter_context(tc.tile_pool(name="load", bufs=4))
        tmpp = ctx.enter_context(tc.tile_pool(name="tmp", bufs=2))

        sc5 = pers.tile([KP, L], F32, tag="sc5")     # 5e-3 * R (A''-scan in)
        sq10 = pers.tile([KP, L], F32, tag="sq10")   # (10R)^2  (B'-scan in)
        R2h = pers.tile([KP, L], F16, tag="R2h")     # (100R)^2
        R16 = pers.tile([KP, L], F16, tag="R16")     # R
        Aloc = pers.tile([KP, LE], F32, tag="Aloc")
        Bloc = pers.tile([KP, LE], F32, tag="Bloc")
        Ap16 = pers.tile([KP, L], F16, tag="Ap16")   # A'' prev (carried)
        Bp16 = pers.tile([KP, L], F16, tag="Bp16")   # B'  prev (carried)
        cvec = pers.tile([KP, KCMAX], F32, tag="cvec")
        geoc = pers.tile([KP, L], F16, tag="geoc")
        initAB = pers.tile([KP, 2], F32, tag="initAB")
        t16a = pers.tile([KP, L], F16, tag="t16a")
        t16b = pers.tile([KP, L], F16, tag="t16b")
        t16c = pers.tile([KP, L], F16, tag="t16c")
        tf32a = pers.tile([KP, L], F32, tag="tf32a")
        tf32b = pers.tile([KP, L], F32, tag="tf32b")
        qsum = pers.tile([KP, 8], F32, tag="qsum")

        # chunk loads first: the SDMA stream is the critical resource
        tiles = []
        off = 0
        for k, kc in enumerate(CHUNKS):
            fw = kc * NA
            wt = loadp.tile([KP, KCMAX * NA], F16, tag="wt")
            rt = loadp.tile([KP, KCMAX * NA], F16, tag="rt")
            nc.gpsimd.dma_start(wt[:, 0:fw], w_v[:, off * NA:(off + kc) * NA])
            nc.gpsimd.dma_start(rt[:, 0:fw], nr_v[:, off * NA:(off + kc) * NA])
            tiles.append((off, kc, wt, rt))
            off += kc

        # constants / scan seeds (DVE/ACT; do not block the DMA stream)
        nc.vector.memset(qsum[0:1, 0:1], 1.0)
        nc.scalar.sqrt(qsum[0:1, 0:1], qsum[0:1, 0:1])  # pin ACT table early
        nc.vector.memset(cvec[:, :], float(CDEC))
        nc.vector.memset(Aloc[:, 0:1], 0.0)
        nc.vector.memset(Bloc[:, 0:1], 0.0)
        nc.vector.memset(initAB[:, :], 0.0)
        nc.scalar.dma_start(geoc[:], geo_dram.ap())

        def negn_chain(h):
            nc.vector.tensor_sub(t16b[:, h], Ap16[:, h], R16[:, h])   # u
            nc.vector.tensor_mul(t16c[:, h], Bp16[:, h], t16b[:, h])  # m1
            nc.vector.tensor_mul(t16b[:, h], Ap16[:, h], R2h[:, h])   # m2
            nc.vector.tensor_add(t16c[:, h], t16c[:, h], t16b[:, h])  # negn4
            nc.vector.tensor_sub(t16b[:, h], Bp16[:, h], t16a[:, h])  # var4
            nc.scalar.sqrt(t16a[:, h], t16b[:, h])                    # s

        def var_chain(h, col):
            nc.vector.tensor_mul(tf32a[:, h], t16b[:, h], t16a[:, h])  # var4^1.5
            nc.vector.reciprocal_approx_fast(tf32b[:, h], tf32a[:, h])
            nc.vector.scalar_tensor_tensor(                            # qsum+=negn*rec
                out=tf32a[:, h], in0=t16c[:, h], scalar=1.0, in1=tf32b[:, h],
                op0=AL.mult, op1=AL.mult, accum_out=qsum[:, col],
            )

        # zero-carry D-chain for a chunk with off >= 1000: there the carry
        # term init*c^t is below fp16 resolution, so Aloc/Bloc are exact
        # enough and the chain can run inside stage-A DMA slack.
        def chain_nocarry(off, kc, col):
            h = slice(off, off + kc)
            nc.vector.tensor_copy(Ap16[:, h], Aloc[:, off:off + kc])
            nc.scalar.activation(t16a[:, h], Ap16[:, h], AF.Square, scale=200.0)
            nc.vector.tensor_copy(Bp16[:, h], Bloc[:, off:off + kc])
            negn_chain(h)
            var_chain(h, col)

        # ---- stage A: product, 16->1 tree, derived streams, scans ----
        qcol = 2
        for off, kc, wt, rt in tiles:
            fw = kc * NA
            ks = slice(off, off + kc)
            prod = tmpp.tile([KP, KCMAX * NA], F16, tag="prod")
            s1 = tmpp.tile([KP, KCMAX * 8], F16, tag="s1")
            s2 = tmpp.tile([KP, KCMAX * 4], F16, tag="s2")
            s3 = tmpp.tile([KP, KCMAX * 2], F16, tag="s3")
            nc.vector.tensor_mul(prod[:, 0:fw], wt[:, 0:fw], rt[:, 0:fw])
            p3 = prod[:, 0:fw].rearrange("p (t a) -> p t a", a=NA)
            s1v = s1[:, 0:kc * 8].rearrange("p (t a) -> p t a", a=8)
            s2v = s2[:, 0:kc * 4].rearrange("p (t a) -> p t a", a=4)
            s3v = s3[:, 0:kc * 2].rearrange("p (t a) -> p t a", a=2)
            nc.vector.tensor_add(s1v[:, :, :], p3[:, :, 0:8], p3[:, :, 8:16])
            nc.vector.tensor_add(s2v[:, :, :], s1v[:, :, 0:4], s1v[:, :, 4:8])
            nc.vector.tensor_add(s3v[:, :, :], s2v[:, :, 0:2], s2v[:, :, 2:4])
            # last tree stage emits f32 R
            nc.vector.tensor_add(tf32a[:, ks], s3v[:, :, 0], s3v[:, :, 1])
            # derived streams on ACT
            nc.scalar.mul(sc5[:, ks], tf32a[:, ks], 5e-3)
            nc.scalar.activation(sq10[:, ks], tf32a[:, ks], AF.Square, scale=10.0)
            nc.scalar.activation(R2h[:, ks], tf32a[:, ks], AF.Square, scale=100.0)
            nc.scalar.activation(R16[:, ks], tf32a[:, ks], AF.Copy)
            # chained scans: state = c*state + x
            nc.vector.tensor_tensor_scan(
                out=Aloc[:, 1 + off:1 + off + kc], data0=cvec[:, 0:kc],
                data1=sc5[:, ks], initial=Aloc[:, off:off + 1],
                op0=AL.mult, op1=AL.add,
            )
            nc.vector.tensor_tensor_scan(
                out=Bloc[:, 1 + off:1 + off + kc], data0=cvec[:, 0:kc],
                data1=sq10[:, ks], initial=Bloc[:, off:off + 1],
                op0=AL.mult, op1=AL.add,
            )
            # inline only where DMA cadence still has DVE slack; the last
            # (tapered) chunks defer their chains to the tail where they
            # overlap the carry shift-DMAs
            if CUT <= off < 1750:
                chain_nocarry(off, kc, slice(qcol, qcol + 1))
                qcol += 1

        # ---- tail ----
        # carries: previous partition's local final (c^2000 kills older terms)
        nc.sync.dma_start(initAB[1:KP, 0:1], Aloc[0:KP - 1, L:LE])
        nc.scalar.dma_start(initAB[1:KP, 1:2], Bloc[0:KP - 1, L:LE])

        # deferred zero-carry chains for the tapered last chunks; these
        # need no carry so they run while the shift-DMAs are in flight
        for off, kc, _, _ in tiles:
            if off >= 1750:
                chain_nocarry(off, kc, slice(qcol, qcol + 1))
                qcol += 1

        def corr(h):
            # A''_prev[:,t] = Aloc[:,t] + init*c^t (Aloc col0 is the zero pad)
            nc.vector.scalar_tensor_tensor(
                out=Ap16[:, h], in0=geoc[:, h], scalar=initAB[:, 0:1],
                in1=Aloc[:, h], op0=AL.mult, op1=AL.add,
            )
            # ACT square can start for this half right away
            nc.scalar.activation(t16a[:, h], Ap16[:, h], AF.Square, scale=200.0)
            nc.vector.scalar_tensor_tensor(
                out=Bp16[:, h], in0=geoc[:, h], scalar=initAB[:, 1:2],
                in1=Bloc[:, h], op0=AL.mult, op1=AL.add,
            )

        # carried pass over cols [0, CUT) only, two pieces interleaved to
        # hide ACT under DVE
        q0 = slice(0, CUT // 2)
        q1 = slice(CUT // 2, CUT)
        corr(q0)
        negn_chain(q0)
        corr(q1)
        var_chain(q0, slice(0, 1))
        negn_chain(q1)
        var_chain(q1, slice(1, 2))

        # per-partition partials ship out; the host reduces (p0 excluded)
        nc.sync.dma_start(out_ap[:, :], qsum[:, :])

    nc.compile()
    return nc


def _get_program():
    global _PROGRAM
    if _PROGRAM is None:
        _PROGRAM = _build_program()
    return _PROGRAM


def _core0_prepend():
    """2000 synthetic rows encoding the global init (A,B)=(0,EPS).

    All-zero rows leave the scan at (0,0); the last two rows carry returns
    r1, r2 with r2 = -fl(c*r1) so the A-scan cancels to ~0, while
    eta*(c*r1^2 + r2^2) ~ EPS supplies the B carry.
    """
    w = np.zeros((L, NA), NF32)
    nr = np.zeros((L, NA), NF32)
    c = CDEC
    r1 = NF32(np.sqrt(EPS / (ETA * (float(c) + float(c) ** 2))))
    r2 = NF32(-(c * r1))
    w[L - 2, 0] = NF32(1.0)
    nr[L - 2, 0] = r1
    w[L - 1, 0] = NF32(1.0)
    nr[L - 1, 0] = r2
    return w, nr


def _make_in_maps(weights, nr):
    weights = np.ascontiguousarray(weights, dtype=NF32)
    nr = np.ascontiguousarray(nr, dtype=NF32)
    pre_w, pre_nr = _core0_prepend()
    in_maps = []
    for m in range(N_CORES):
        s = m * OWN
        if m == 0:
            wm = np.concatenate([pre_w, weights[:OWN]])
            rm = np.concatenate([pre_nr, nr[:OWN]])
        else:
            wm = weights[s - L:s + OWN]
            rm = nr[s - L:s + OWN]
        in_maps.append({"w": wm, "nr": rm})
    return in_maps


def _run(in_maps, **kwargs):
    nc = _get_program()
    return run_bass_kernel_spmd(nc, in_maps, core_ids=list(range(N_CORES)), **kwargs)


def kernel(weights, next_returns):
    in_maps = _make_in_maps(weights, next_returns)
    res = _run(in_maps)
    total = NF32(0.0)
    for m in range(N_CORES):
        q = np.asarray(res.results[m]["out"], NF32)
        total = NF32(total + np.sum(q[1:, :], dtype=NF32))
    # q = 1e-2 * negn/var^1.5 and loss = eta*sum(negn/var^1.5)/B, eta*100 = 1
    return NF32(total / NF32(B_TOTAL))
